# revision 20
# baseline (speedup 1.0000x reference)
"""Trainium2 Bass kernel for NodeTimeSeriesDecoder (per-node 2-layer LSTM over T=256).

Sharding: data-parallel over graphs across 8 cores (whole graphs -> contiguous
node blocks). Per core nodes are padded per-graph to multiples of B=1024 and
processed as independent per-block LSTM chains; NI chains are interleaved in
one time loop so all engines stay busy despite the per-chain serial dependency.

The activation engine (ScalarE) is the bottleneck (10 nonlinear passes per
block-step saturate it). This version moves all tanh evaluations (g-gates and
tanh(c)) onto the Vector engine as a fused custom DVE op: a monic degree-7 odd
polynomial (8 ALU stages exactly). The per-(graph,t) gate bias rides the op's
Src1 [P,1] broadcast; the polynomial's leading-coefficient normalization is
folded into the host-side g-gate weight rows (scale s_G) and into a rescaled
cell state c_hat = s_C * c (head weights unscale it). Sigmoid gates remain on
ScalarE; the f*c products run on the otherwise-idle GPSIMD engine.
"""
import sys
sys.path.insert(0, "/opt/trn_rl_repo")
import re
import numpy as np
import ml_dtypes
import concourse.bass as bass
import concourse.bacc as bacc
import concourse.tile as tile
from concourse import mybir
from concourse.bass_utils import run_bass_kernel_spmd


F32 = mybir.dt.float32
F32R = mybir.dt.float32r
BF16 = mybir.dt.bfloat16
AF = mybir.ActivationFunctionType
ALU = mybir.AluOpType
DS = bass.DynSlice

H = 128
T = 256
GM = 3
ND = 6
B = 1024
NCORES = 8
NI = 4        # interleaved independent block chains
UNROLL = 8
P0 = 6
THALF = T // 2
NCH = B // H

# ---- degree-7 odd minimax-ish tanh fits (weighted toward the data bulk) ----
# tanh(x) ~= x*(a0 + a1 x^2 + a2 x^4 + a3 x^6) on |x| <= B_fit
A_G = (0.980187350165812, -0.2583327462684275, 0.0475225918234586, -0.00347542814802112)   # B_fit=2.45 (gate preacts)
A_C = (0.9904704872618362, -0.2862214716966526, 0.06479055237507263, -0.0063245456932585955)  # B_fit=2.0 (cell state)


def _monic(a):
    # y = s*x; tanh(x) ~= (((b2 - u)*u + b1)*u + b0) * y, u = y^2
    s = float((-a[3]) ** (1.0 / 7.0))
    b0 = a[0] / s
    b1 = a[1] / s ** 3
    b2 = a[2] / s ** 5
    return s, float(b0), float(b1), float(b2)


S_C, B0_C, B1_C, B2_C = _monic(A_C)
# g-site poly outputs s_C * tanh(x) directly (so i*g products are plain muls)
_A_GS = tuple(S_C * a for a in A_G)
S_G, B0_G, B1_G, B2_G = _monic(_A_GS)

# ---- custom DVE ops: monic degree-7 tanh on pre-scaled input ---------------
# NTANH7B: out = poly7(Src0 + latch(Src1))  (per-partition bias, for g-gates)
# NTANH7P: out = poly7(Src0)                (for tanh(c_hat))
_TANH_OPS = None


def _register_op(DO, name, spec):
    for existing in DO.OPS:
        if existing.name == name:
            return existing
    op = DO.DveOp(name, spec, subdim=False, uops_sha={})
    DO.OPS.append(op)
    DO.CUSTOM_DVE_SPECS[op.name] = op.spec
    DO._SUB_OPCODE_FOR_NAME[op.name] = DO._CUSTOM_DVE_ROW_BASE + len(DO.OPS) - 1
    assert DO._SUB_OPCODE_FOR_NAME[op.name] < 0x20
    for ver in ("v3", "v4"):
        try:
            op.compile(ver)
        except ValueError as e:
            m = re.search(r'="([0-9a-f]+)"', str(e))
            if not m:
                raise
            op.uops_sha[ver] = m.group(1)
            DO._COMPILE_CACHE.pop((op.name, ver), None)
            op.compile(ver)
    return op


def _get_tanh_ops():
    global _TANH_OPS
    if _TANH_OPS is not None:
        return _TANH_OPS
    from concourse import dve_ops as DO
    from concourse.dve_spec import Spec, Src0, Src1, C0, C1, C2, sq, Latch

    def _refb(in0, in1, s0, s1, imm2):
        tt = in0.astype(np.float32) + in1
        uu = tt * tt
        return (((s0 - uu) * uu + s1) * uu + imm2) * tt

    def _refp(in0, s0, s1, imm2):
        tt = in0.astype(np.float32)
        uu = tt * tt
        return (((s0 - uu) * uu + s1) * uu + imm2) * tt

    tb = Src0 + Latch(Src1)
    ub = sq(tb)
    opb = _register_op(DO, "NTANH7B",
                       Spec(body=(((C0 - ub) * ub + C1) * ub + C2) * tb,
                            reference=_refb))
    tp = Src0
    up = sq(tp)
    opp = _register_op(DO, "NTANH7P",
                       Spec(body=(((C0 - up) * up + C1) * up + C2) * tp,
                            reference=_refp))
    _TANH_OPS = (opb, opp)
    return _TANH_OPS


def build_nc(NBLK, NTAB, blkmap):
    NPAD = NBLK * B
    TANH7B, TANH7P = _get_tanh_ops()
    nc = bacc.Bacc(None, target_bir_lowering=False)

    node_t_ext = nc.declare_dram_parameter("node_t", [ND, NPAD], F32R, isOutput=False)
    m12_ext = nc.declare_dram_parameter("m12", [H, NTAB, 8, T], BF16, isOutput=False)
    e0_ext = nc.declare_dram_parameter("e0", [H, NTAB], F32, isOutput=False)
    e0s_ext = nc.declare_dram_parameter("e0s", [H, NTAB], F32, isOutput=False)
    wencT_node_ext = nc.declare_dram_parameter("wencT_node", [ND, H], F32R, isOutput=False)
    wihT0_ext = nc.declare_dram_parameter("wihT0", [H, 4 * H], BF16, isOutput=False)
    whhT0_ext = nc.declare_dram_parameter("whhT0", [H, 4 * H], BF16, isOutput=False)
    wihT1_ext = nc.declare_dram_parameter("wihT1", [H, 4 * H], BF16, isOutput=False)
    whhT1_ext = nc.declare_dram_parameter("whhT1", [H, 4 * H], BF16, isOutput=False)
    wd1T_ext = nc.declare_dram_parameter("wd1T", [H, 2, 64], BF16, isOutput=False)
    wd2T_ext = nc.declare_dram_parameter("wd2T", [64, 4], BF16, isOutput=False)
    bd1_ext = nc.declare_dram_parameter("bd1", [64, 1], F32, isOutput=False)
    bd2rep_ext = nc.declare_dram_parameter("bd2rep", [H, GM], F32, isOutput=False)
    y_ext = nc.declare_dram_parameter("y", [NPAD, T, GM], F32, isOutput=True)

    GATE_SIG = {0: AF.Sigmoid, 1: AF.Sigmoid, 3: AF.Sigmoid}

    with tile.TileContext(nc) as tc:
        with tc.tile_pool(name="consts", bufs=1) as cp, \
             tc.tile_pool(name="work", bufs=1) as wp, \
             tc.tile_pool(name="ps", bufs=8, space="PSUM") as pp:

            wencT_node = cp.tile([ND, H], F32R)
            wihT0 = cp.tile([H, 4 * H], BF16)
            whhT0 = cp.tile([H, 4 * H], BF16)
            wihT1 = cp.tile([H, 4 * H], BF16)
            whhT1 = cp.tile([H, 4 * H], BF16)
            wd1T = cp.tile([H, 2, 64], BF16)
            wd2T = cp.tile([64, 4], BF16)
            bd1 = cp.tile([64, 1], F32)
            bd2rep = cp.tile([H, GM], F32)
            m12 = cp.tile([H, NTAB, 8, T], BF16)
            e0 = cp.tile([H, NTAB], F32)
            e0s = cp.tile([H, NTAB], F32)
            zcol = cp.tile([H, 1], F32)
            nc.vector.memset(zcol, 0.0)
            for dst, src in [(wencT_node, wencT_node_ext), (wihT0, wihT0_ext),
                             (whhT0, whhT0_ext), (wihT1, wihT1_ext), (whhT1, whhT1_ext),
                             (wd1T, wd1T_ext), (wd2T, wd2T_ext), (bd1, bd1_ext),
                             (bd2rep, bd2rep_ext), (m12, m12_ext), (e0, e0_ext),
                             (e0s, e0s_ext)]:
                nc.sync.dma_start(out=dst, in_=src[:])

            HB = 512  # matmul free-dim chunk
            NHB = B // HB

            for bg in range(0, NBLK, NI):
                blocks = list(range(bg, min(bg + NI, NBLK)))
                ctx = {}
                NPAIR = (len(blocks) + 1) // 2
                pairs = {}
                for q in range(NPAIR):
                    pairs[q] = dict(
                        enc=wp.tile([H, 2 * B], BF16, tag=f"encn{q}", bufs=1, name=f"encn{q}"),
                        h1=[wp.tile([H, 2 * B], BF16, tag=f"h1_{q}{p}", bufs=1, name=f"h1_{q}{p}") for p in range(2)],
                        c1=[wp.tile([H, 2 * B], BF16, tag=f"c1_{q}{p}", bufs=1, name=f"c1_{q}{p}") for p in range(2)],
                        h2=[wp.tile([H, 2 * B], BF16, tag=f"h2_{q}{p}", bufs=1, name=f"h2_{q}{p}") for p in range(2)],
                        c2=[wp.tile([H, 2 * B], BF16, tag=f"c2_{q}{p}", bufs=1, name=f"c2_{q}{p}") for p in range(2)],
                        si=[wp.tile([H, 2 * B], BF16, tag=f"si_{q}", bufs=2, name=f"si_{q}_{l}") for l in range(2)],
                        sf=[wp.tile([H, 2 * B], BF16, tag=f"sf_{q}", bufs=2, name=f"sf_{q}_{l}") for l in range(2)],
                        so=[wp.tile([H, 2 * B], BF16, tag=f"so_{q}", bufs=2, name=f"so_{q}_{l}") for l in range(2)],
                        tg=[wp.tile([H, 2 * B], BF16, tag=f"tg_{q}", bufs=2, name=f"tg_{q}_{l}") for l in range(2)],
                        stg=[wp.tile([H, 8], F32, tag=f"stg{q}{p}", bufs=1, name=f"stg{q}{p}") for p in range(2)],
                    )
                for i, b in enumerate(blocks):
                    q, hf = i // 2, i % 2
                    P = pairs[q]
                    sl_h = slice(hf * B, (hf + 1) * B)
                    nb = wp.tile([ND, B], F32R, tag="nb", bufs=2, name=f"nb{b}")
                    nc.sync.dma_start(out=nb, in_=node_t_ext[:, b * B:(b + 1) * B])
                    px = pp.tile([H, B], F32, tag="psg", bufs=2, name=f"px{b}")
                    for hh in range(NHB):
                        nc.tensor.matmul(px[:, hh * HB:(hh + 1) * HB], wencT_node,
                                         nb[:, hh * HB:(hh + 1) * HB], start=True, stop=True)
                    nc.vector.tensor_copy(P["enc"][:, sl_h], px)
                    for dst in (P["h1"][0], P["h2"][0]):
                        nc.vector.tensor_scalar_add(dst[:, sl_h], px, e0[:, blkmap[b]:blkmap[b] + 1])
                    for dst in (P["c1"][0], P["c2"][0]):
                        nc.scalar.activation(out=dst[:, sl_h], in_=px, func=AF.Identity,
                                             bias=e0s[:, blkmap[b]:blkmap[b] + 1], scale=S_C)
                    ctx[i] = dict(b=b, g=blkmap[b], q=q, hf=hf)

                def gates(i, l, whhT, wihT, hprev, xin, stg_p, k_in):
                    """4 gate psums -> sig/tanh results into pair-tile halves."""
                    q, hf = ctx[i]["q"], ctx[i]["hf"]
                    P = pairs[q]
                    sl_h = slice(hf * B, (hf + 1) * B)
                    dests = [P["si"][l], P["sf"][l], P["tg"][l], P["so"][l]]
                    for c in range(4):
                        pg = pp.tile([H, B], F32, tag="psg" if c == 2 else "ps2",
                                     bufs=2, name=f"g_{c}")
                        for hh in range(NHB):
                            sl = slice(hh * HB, (hh + 1) * HB)
                            nc.tensor.matmul(pg[:, sl], whhT[:, c * H:(c + 1) * H],
                                             hprev[:, sl], start=True, stop=False)
                        for hh in range(NHB):
                            sl = slice(hh * HB, (hh + 1) * HB)
                            nc.tensor.matmul(pg[:, sl], wihT[0:k_in, c * H:(c + 1) * H],
                                             xin[0:k_in, sl], start=False, stop=True)
                        a = dests[c][:, sl_h]
                        if c == 2:
                            nc.vector._custom_dve(TANH7B, out=a, in0=pg,
                                                  in1=stg_p[:, 2:3],
                                                  s0=B2_G, s1=B1_G, imm2=B0_G)
                        else:
                            nc.scalar.activation(out=a, in_=pg, func=GATE_SIG[c],
                                                 bias=stg_p[:, c:c + 1], scale=1.0)

                def stage1g(i, t, par):
                    q, hf = ctx[i]["q"], ctx[i]["hf"]
                    P = pairs[q]
                    pcur = par
                    if hf == 0:
                        nc.gpsimd.tensor_copy(P["stg"][pcur], m12[:, ctx[i]["g"], :, DS(t, 1)].squeeze(-1))
                    sl_h = slice(hf * B, (hf + 1) * B)
                    gates(i, 0, whhT0, wihT0, P["h1"][pcur][:, sl_h], P["enc"][:, sl_h],
                          P["stg"][pcur], H)

                def stage1c(q, par):
                    P = pairs[q]
                    pcur, pnxt = par, 1 - par
                    t1 = wp.tile([H, 2 * B], BF16, tag="prodB", bufs=4, name="t1")
                    t2 = wp.tile([H, 2 * B], BF16, tag="prodB", bufs=4, name="t2")
                    nc.vector.tensor_mul(t1, P["si"][0], P["tg"][0])
                    nc.vector.tensor_mul(t2, P["sf"][0], P["c1"][pcur])
                    nc.vector.tensor_add(P["c1"][pnxt], t1, t2)

                def stage1b(q, par):
                    P = pairs[q]
                    pnxt = 1 - par
                    tc1 = wp.tile([H, 2 * B], BF16, tag="actsB", bufs=4, name="tc1")
                    nc.vector._custom_dve(TANH7P, out=tc1, in0=P["c1"][pnxt],
                                          s0=B2_C, s1=B1_C, imm2=B0_C)
                    nc.vector.tensor_mul(P["h1"][pnxt], P["so"][0], tc1)

                def stage2g(i, par):
                    q, hf = ctx[i]["q"], ctx[i]["hf"]
                    P = pairs[q]
                    pcur, pnxt = par, 1 - par
                    sl_h = slice(hf * B, (hf + 1) * B)
                    stg2 = P["stg"][pcur][:, 4:]
                    gates(i, 1, whhT1, wihT1, P["h2"][pcur][:, sl_h],
                          P["h1"][pnxt][:, sl_h], stg2, H - GM)

                def stage2c(q, par):
                    P = pairs[q]
                    pcur, pnxt = par, 1 - par
                    t3 = wp.tile([H, 2 * B], BF16, tag="prodB", bufs=4, name="t3")
                    t4 = wp.tile([H, 2 * B], BF16, tag="prodB", bufs=4, name="t4")
                    nc.vector.tensor_mul(t3, P["si"][1], P["tg"][1])
                    nc.vector.tensor_mul(t4, P["sf"][1], P["c2"][pcur])
                    nc.vector.tensor_add(P["c2"][pnxt], t3, t4)

                def stage2b(q, par):
                    P = pairs[q]
                    pnxt = 1 - par
                    tc2 = wp.tile([H, 2 * B], BF16, tag="actsB", bufs=4, name="tc2")
                    nc.vector._custom_dve(TANH7P, out=tc2, in0=P["c2"][pnxt],
                                          s0=B2_C, s1=B1_C, imm2=B0_C)
                    nc.vector.tensor_mul(P["h2"][pnxt], P["so"][1], tc2)

                def stage3(i, th, par, half):
                    b = ctx[i]["b"]
                    q, hf = ctx[i]["q"], ctx[i]["hf"]
                    P = pairs[q]
                    h2 = P["h2"][1 - par][:, hf * B:(hf + 1) * B]
                    c2 = P["c2"][1 - par][:, hf * B:(hf + 1) * B]
                    hd = pp.tile([H, B], F32, tag="psg", bufs=2, name="hd")
                    pd = hd[0:64, :]
                    for hh in range(NHB):
                        sl = slice(hh * HB, (hh + 1) * HB)
                        nc.tensor.matmul(pd[:, sl], wd1T[:, 0, :], h2[:, sl],
                                         start=True, stop=False)
                    for hh in range(NHB):
                        sl = slice(hh * HB, (hh + 1) * HB)
                        nc.tensor.matmul(pd[:, sl], wd1T[:, 1, :], c2[:, sl],
                                         start=False, stop=True)
                    relu = wp.tile([64, B], BF16, tag="relu", bufs=3, name="relu")
                    nc.scalar.activation(out=relu, in_=pd, func=AF.Relu,
                                         bias=bd1, scale=1.0)
                    py = hd[:, 0:4 * NCH]
                    for j in range(NCH):
                        nc.tensor.matmul(py[:, j * 4:(j + 1) * 4],
                                         relu[:, j * H:(j + 1) * H], wd2T,
                                         start=True, stop=True)
                    ybuf = wp.tile([H, NCH, GM], F32, tag="ybuf", bufs=4, name="ybuf")
                    nc.scalar.copy(
                        ybuf, py.rearrange("p (j four) -> p j four", four=4)[:, :, 0:GM])
                    nc.sync.dma_start(
                        out=y_ext[b * B:(b + 1) * B, DS(th + half * THALF, 1), :]
                            .rearrange("(j p) t g -> p j (t g)", p=H),
                        in_=ybuf)

                NB_ = len(blocks)

                def S1(s_expr, par, half):
                    for i in range(NB_):
                        stage1g(i, s_expr + half * THALF, par)
                    for q in range(NPAIR):
                        stage1c(q, par)
                    for q in range(NPAIR):
                        stage1b(q, par)

                def S2(par):
                    for i in range(NB_):
                        stage2g(i, par)
                    for q in range(NPAIR):
                        stage2c(q, par)
                    for q in range(NPAIR):
                        stage2b(q, par)

                def S3(th_expr, par, half):
                    for i in range(NB_):
                        stage3(i, th_expr, par, half)

                assert (THALF - 2 - P0) % UNROLL == 0 and P0 % 2 == 0
                for half in range(2):
                    for sp in range(P0):
                        S1(sp, sp % 2, half)
                        if sp >= 1: S2((sp - 1) % 2)
                        if sp >= 2: S3(sp - 2, sp % 2, half)
                    with tc.For_i(P0, THALF - 2, UNROLL,
                                  hint_engines=(mybir.EngineType.PE,
                                                mybir.EngineType.Activation,
                                                mybir.EngineType.DVE)) as tv:
                        for k in range(UNROLL):
                            par = k % 2            # (tv + k) % 2, tv even
                            S1(tv + k, par, half)
                            S2(1 - par)            # step tv+k-1
                            S3(tv + k - 2, par, half)  # step tv+k-2
                    for sp in range(THALF - 2, THALF):
                        S1(sp, sp % 2, half)
                        S2((sp - 1) % 2)
                        S3(sp - 2, sp % 2, half)
                    S2(1)
                    S3(THALF - 2, 0, half)
                    S3(THALF - 1, 1, half)

    nc.finalize()
    return nc


_CACHE = {}
_LAST_IN_MAPS = None


def _get_nc(NBLK, NTAB, blkmap):
    key = (NBLK, NTAB, tuple(blkmap))
    if key not in _CACHE:
        _CACHE[key] = build_nc(NBLK, NTAB, blkmap)
    return _CACHE[key]


def kernel(node, ptr, graph_time_series_behavior, ground_motions,
           W_enc, b_enc, W_ih, W_hh, b_ih, b_hh, W_d1, b_d1, W_d2, b_d2):
    node = np.asarray(node, np.float32)
    ptr = np.asarray(ptr, np.int64)
    lat = np.asarray(graph_time_series_behavior, np.float32)
    gms = np.asarray(ground_motions, np.float32)
    W_enc = np.asarray(W_enc, np.float32); b_enc_a = np.asarray(b_enc, np.float32)
    W_ih = np.array(W_ih, np.float32, copy=True); W_hh = np.array(W_hh, np.float32, copy=True)
    b_ih = np.array(b_ih, np.float32, copy=True); b_hh = np.array(b_hh, np.float32, copy=True)
    W_d1 = np.array(W_d1, np.float32, copy=True); b_d1_a = np.asarray(b_d1, np.float32)
    W_d2 = np.asarray(W_d2, np.float32); b_d2_a = np.asarray(b_d2, np.float32)

    # fold the tanh-poly monic normalization into the weights:
    #  - g-gate rows (2H:3H) of both layers scaled by s_G (device computes
    #    tanh via the monic poly on the pre-scaled preactivation)
    #  - cell state stored as c_hat = s_C * c  => head's c columns unscale
    for l in range(2):
        W_ih[l][2 * H:3 * H, :] *= S_G
        W_hh[l][2 * H:3 * H, :] *= S_G
        b_ih[l][2 * H:3 * H] *= S_G
        b_hh[l][2 * H:3 * H] *= S_G
    W_d1[:, H:] /= S_C

    N = node.shape[0]
    BS = lat.shape[0]
    gsizes = np.diff(ptr).astype(np.int64)
    assert gsizes.sum() == N

    gper = (BS + NCORES - 1) // NCORES
    core_graphs = [list(range(c * gper, min((c + 1) * gper, BS))) for c in range(NCORES)]
    NTAB = max(len(cg) for cg in core_graphs)
    core_blkmaps, core_nblk = [], []
    for cg in core_graphs:
        bm = []
        for slot, g in enumerate(cg):
            bm += [slot] * int((gsizes[g] + B - 1) // B)
        core_blkmaps.append(bm)
        core_nblk.append(len(bm))
    NBLK = max(core_nblk) if max(core_nblk) > 0 else 1
    for bm in core_blkmaps:
        bm += [0] * (NBLK - len(bm))
    if all(bm == core_blkmaps[0] for bm in core_blkmaps):
        blkmap = core_blkmaps[0]
        per_block_tabs = False
    else:
        blkmap = list(range(NBLK))
        NTAB = NBLK
        per_block_tabs = True

    NPAD = NBLK * B

    # host-precomputed per-(graph, t) tables (small); computed AFTER the
    # s_G scaling above so g-gate biases arrive pre-scaled too
    enc_mix = np.einsum("hk,gtk->gth", W_enc[:, ND:ND + H], lat) \
        + np.einsum("hk,gtk->gth", W_enc[:, ND + H:], gms) + b_enc_a
    m1 = np.einsum("rh,gth->gtr", W_ih[0], enc_mix) + (b_ih[0] + b_hh[0])
    m2 = np.einsum("rk,gtk->gtr", W_ih[1][:, H - GM:], gms) + (b_ih[1] + b_hh[1])
    m12_full = np.concatenate([m1.reshape(BS, T, 4, H), m2.reshape(BS, T, 4, H)], axis=2)
    m12_full = np.ascontiguousarray(m12_full.transpose(3, 0, 2, 1))  # [H, BS, 8, T]
    e0_full = np.ascontiguousarray(enc_mix[:, 0, :].T)               # [H, BS]

    weights_common = dict(
        wencT_node=np.ascontiguousarray(W_enc[:, :ND].T),
        wihT0=np.ascontiguousarray(W_ih[0].T).astype(ml_dtypes.bfloat16),
        whhT0=np.ascontiguousarray(W_hh[0].T).astype(ml_dtypes.bfloat16),
        wihT1=np.ascontiguousarray(W_ih[1].T).astype(ml_dtypes.bfloat16),
        whhT1=np.ascontiguousarray(W_hh[1].T).astype(ml_dtypes.bfloat16),
        wd1T=np.ascontiguousarray(np.stack([W_d1[:, :H].T, W_d1[:, H:].T], axis=1)).astype(ml_dtypes.bfloat16),
        wd2T=np.ascontiguousarray(np.concatenate([W_d2.T, np.zeros((64, 1), np.float32)], 1)).astype(ml_dtypes.bfloat16),
        bd1=b_d1_a.reshape(64, 1),
        bd2rep=np.ascontiguousarray(np.broadcast_to(b_d2_a, (H, GM))),
    )

    in_maps, core_index_maps = [], []
    for c, cg in enumerate(core_graphs):
        node_pad = np.zeros((NPAD, ND), np.float32)
        idx_map = np.full(NPAD, -1, np.int64)
        pos = 0
        for g in cg:
            s, e = int(ptr[g]), int(ptr[g + 1])
            n = e - s
            node_pad[pos:pos + n] = node[s:e]
            idx_map[pos:pos + n] = np.arange(s, e)
            pos += int((n + B - 1) // B) * B
        m12_c = np.zeros((H, NTAB, 8, T), np.float32)
        e0_c = np.zeros((H, NTAB), np.float32)
        if per_block_tabs:
            bi = 0
            for g in cg:
                for _ in range(int((gsizes[g] + B - 1) // B)):
                    m12_c[:, bi] = m12_full[:, g]
                    e0_c[:, bi] = e0_full[:, g]
                    bi += 1
        else:
            for slot, g in enumerate(cg):
                m12_c[:, slot] = m12_full[:, g]
                e0_c[:, slot] = e0_full[:, g]
        in_maps.append(dict(
            node_t=np.ascontiguousarray(node_pad.T),
            m12=m12_c.astype(ml_dtypes.bfloat16),
            e0=e0_c,
            e0s=(S_C * e0_c),
            **weights_common,
        ))
        core_index_maps.append(idx_map)

    global _LAST_IN_MAPS
    _LAST_IN_MAPS = in_maps
    nc = _get_nc(NBLK, NTAB, blkmap)
    res = run_bass_kernel_spmd(nc, in_maps, list(range(NCORES)))

    out = np.empty((N, T, GM), np.float32)
    for c in range(NCORES):
        y = np.asarray(res.results[c]["y"], dtype=np.float32)
        m = core_index_maps[c]
        valid = m >= 0
        out[m[valid]] = y[valid]
    out += b_d2_a
    return out


# revision 21
# speedup vs baseline: 1.0266x; 1.0266x over previous
"""Trainium2 Bass kernel for NodeTimeSeriesDecoder (per-node 2-layer LSTM over T=256).

Sharding: data-parallel over graphs across 8 cores (whole graphs -> contiguous
node blocks). Per core nodes are padded per-graph to multiples of B=1024 and
processed as independent per-block LSTM chains; NI chains are interleaved in
one time loop so all engines stay busy despite the per-chain serial dependency.

The activation engine (ScalarE) is the bottleneck (10 nonlinear passes per
block-step saturate it). This version moves all tanh evaluations (g-gates and
tanh(c)) onto the Vector engine as a fused custom DVE op: a monic degree-7 odd
polynomial (8 ALU stages exactly). The per-(graph,t) gate bias rides the op's
Src1 [P,1] broadcast; the polynomial's leading-coefficient normalization is
folded into the host-side g-gate weight rows (scale s_G) and into a rescaled
cell state c_hat = s_C * c (head weights unscale it). Sigmoid gates remain on
ScalarE; the f*c products run on the otherwise-idle GPSIMD engine.
"""
import sys
sys.path.insert(0, "/opt/trn_rl_repo")
import re
import numpy as np
import ml_dtypes
import concourse.bass as bass
import concourse.bacc as bacc
import concourse.tile as tile
from concourse import mybir
from concourse.bass_utils import run_bass_kernel_spmd


F32 = mybir.dt.float32
F32R = mybir.dt.float32r
BF16 = mybir.dt.bfloat16
AF = mybir.ActivationFunctionType
ALU = mybir.AluOpType
DS = bass.DynSlice

H = 128
T = 256
GM = 3
ND = 6
B = 1024
NCORES = 8
NI = 4        # interleaved independent block chains
UNROLL = 8
P0 = 6
THALF = T // 2
NCH = B // H

# ---- degree-7 odd minimax-ish tanh fits (weighted toward the data bulk) ----
# tanh(x) ~= x*(a0 + a1 x^2 + a2 x^4 + a3 x^6) on |x| <= B_fit
A_G = (0.980187350165812, -0.2583327462684275, 0.0475225918234586, -0.00347542814802112)   # B_fit=2.45 (gate preacts)
A_C = (0.9904704872618362, -0.2862214716966526, 0.06479055237507263, -0.0063245456932585955)  # B_fit=2.0 (cell state)


def _monic(a):
    # y = s*x; tanh(x) ~= (((b2 - u)*u + b1)*u + b0) * y, u = y^2
    s = float((-a[3]) ** (1.0 / 7.0))
    b0 = a[0] / s
    b1 = a[1] / s ** 3
    b2 = a[2] / s ** 5
    return s, float(b0), float(b1), float(b2)


S_C, B0_C, B1_C, B2_C = _monic(A_C)
# g-site poly outputs s_C * tanh(x) directly (so i*g products are plain muls)
_A_GS = tuple(S_C * a for a in A_G)
S_G, B0_G, B1_G, B2_G = _monic(_A_GS)

# ---- custom DVE ops: monic degree-7 tanh on pre-scaled input ---------------
# NTANH7B: out = poly7(Src0 + latch(Src1))  (per-partition bias, for g-gates)
# NTANH7P: out = poly7(Src0)                (for tanh(c_hat))
_TANH_OPS = None


def _register_op(DO, name, spec):
    for existing in DO.OPS:
        if existing.name == name:
            return existing
    op = DO.DveOp(name, spec, subdim=False, uops_sha={})
    DO.OPS.append(op)
    DO.CUSTOM_DVE_SPECS[op.name] = op.spec
    DO._SUB_OPCODE_FOR_NAME[op.name] = DO._CUSTOM_DVE_ROW_BASE + len(DO.OPS) - 1
    assert DO._SUB_OPCODE_FOR_NAME[op.name] < 0x20
    for ver in ("v3", "v4"):
        try:
            op.compile(ver)
        except ValueError as e:
            m = re.search(r'="([0-9a-f]+)"', str(e))
            if not m:
                raise
            op.uops_sha[ver] = m.group(1)
            DO._COMPILE_CACHE.pop((op.name, ver), None)
            op.compile(ver)
    return op


def _get_tanh_ops():
    global _TANH_OPS
    if _TANH_OPS is not None:
        return _TANH_OPS
    from concourse import dve_ops as DO
    from concourse.dve_spec import Spec, Src0, Src1, C0, C1, C2, sq, Latch

    def _refb(in0, in1, s0, s1, imm2):
        tt = in0.astype(np.float32) + in1
        uu = tt * tt
        return (((s0 - uu) * uu + s1) * uu + imm2) * tt

    def _refp(in0, s0, s1, imm2):
        tt = in0.astype(np.float32)
        uu = tt * tt
        return (((s0 - uu) * uu + s1) * uu + imm2) * tt

    tb = Src0 + Latch(Src1)
    ub = sq(tb)
    opb = _register_op(DO, "NTANH7B",
                       Spec(body=(((C0 - ub) * ub + C1) * ub + C2) * tb,
                            reference=_refb))
    tp = Src0
    up = sq(tp)
    opp = _register_op(DO, "NTANH7P",
                       Spec(body=(((C0 - up) * up + C1) * up + C2) * tp,
                            reference=_refp))
    _TANH_OPS = (opb, opp)
    return _TANH_OPS


def build_nc(NBLK, NTAB, blkmap):
    NPAD = NBLK * B
    TANH7B, TANH7P = _get_tanh_ops()
    nc = bacc.Bacc(None, target_bir_lowering=False)

    node_t_ext = nc.declare_dram_parameter("node_t", [ND, NPAD], F32R, isOutput=False)
    m12_ext = nc.declare_dram_parameter("m12", [H, NTAB, 8, T], BF16, isOutput=False)
    e0_ext = nc.declare_dram_parameter("e0", [H, NTAB], F32, isOutput=False)
    e0s_ext = nc.declare_dram_parameter("e0s", [H, NTAB], F32, isOutput=False)
    wencT_node_ext = nc.declare_dram_parameter("wencT_node", [ND, H], F32R, isOutput=False)
    wihT0_ext = nc.declare_dram_parameter("wihT0", [H, 4 * H], BF16, isOutput=False)
    whhT0_ext = nc.declare_dram_parameter("whhT0", [H, 4 * H], BF16, isOutput=False)
    wihT1_ext = nc.declare_dram_parameter("wihT1", [H, 4 * H], BF16, isOutput=False)
    whhT1_ext = nc.declare_dram_parameter("whhT1", [H, 4 * H], BF16, isOutput=False)
    wd1T_ext = nc.declare_dram_parameter("wd1T", [H, 2, 64], BF16, isOutput=False)
    wd2T_ext = nc.declare_dram_parameter("wd2T", [64, 4], BF16, isOutput=False)
    bd1_ext = nc.declare_dram_parameter("bd1", [64, 1], F32, isOutput=False)
    bd2rep_ext = nc.declare_dram_parameter("bd2rep", [H, GM], F32, isOutput=False)
    y_ext = nc.declare_dram_parameter("y", [NPAD, T, GM], F32, isOutput=True)

    GATE_SIG = {0: AF.Sigmoid, 1: AF.Sigmoid, 3: AF.Sigmoid}

    with tile.TileContext(nc) as tc:
        with tc.tile_pool(name="consts", bufs=1) as cp, \
             tc.tile_pool(name="work", bufs=1) as wp, \
             tc.tile_pool(name="ps", bufs=8, space="PSUM") as pp:

            wencT_node = cp.tile([ND, H], F32R)
            wihT0 = cp.tile([H, 4 * H], BF16)
            whhT0 = cp.tile([H, 4 * H], BF16)
            wihT1 = cp.tile([H, 4 * H], BF16)
            whhT1 = cp.tile([H, 4 * H], BF16)
            wd1T = cp.tile([H, 2, 64], BF16)
            wd2T = cp.tile([64, 4], BF16)
            bd1 = cp.tile([64, 1], F32)
            bd2rep = cp.tile([H, GM], F32)
            m12 = cp.tile([H, NTAB, 8, T], BF16)
            e0 = cp.tile([H, NTAB], F32)
            e0s = cp.tile([H, NTAB], F32)
            zcol = cp.tile([H, 1], F32)
            nc.vector.memset(zcol, 0.0)
            for dst, src in [(wencT_node, wencT_node_ext), (wihT0, wihT0_ext),
                             (whhT0, whhT0_ext), (wihT1, wihT1_ext), (whhT1, whhT1_ext),
                             (wd1T, wd1T_ext), (wd2T, wd2T_ext), (bd1, bd1_ext),
                             (bd2rep, bd2rep_ext), (m12, m12_ext), (e0, e0_ext),
                             (e0s, e0s_ext)]:
                nc.sync.dma_start(out=dst, in_=src[:])

            HB = 512  # matmul free-dim chunk
            NHB = B // HB

            for bg in range(0, NBLK, NI):
                blocks = list(range(bg, min(bg + NI, NBLK)))
                ctx = {}
                NPAIR = (len(blocks) + 1) // 2
                pairs = {}
                for q in range(NPAIR):
                    pairs[q] = dict(
                        enc=wp.tile([H, 2 * B], BF16, tag=f"encn{q}", bufs=1, name=f"encn{q}"),
                        h1=[wp.tile([H, 2 * B], BF16, tag=f"h1_{q}{p}", bufs=1, name=f"h1_{q}{p}") for p in range(2)],
                        c1=[wp.tile([H, 2 * B], BF16, tag=f"c1_{q}{p}", bufs=1, name=f"c1_{q}{p}") for p in range(2)],
                        h2=[wp.tile([H, 2 * B], BF16, tag=f"h2_{q}{p}", bufs=1, name=f"h2_{q}{p}") for p in range(2)],
                        c2=[wp.tile([H, 2 * B], BF16, tag=f"c2_{q}{p}", bufs=1, name=f"c2_{q}{p}") for p in range(2)],
                        si=[wp.tile([H, 2 * B], BF16, tag=f"si_{q}", bufs=2, name=f"si_{q}_{l}") for l in range(2)],
                        sf=[wp.tile([H, 2 * B], BF16, tag=f"sf_{q}", bufs=2, name=f"sf_{q}_{l}") for l in range(2)],
                        so=[wp.tile([H, 2 * B], BF16, tag=f"so_{q}", bufs=2, name=f"so_{q}_{l}") for l in range(2)],
                        tg=[wp.tile([H, 2 * B], BF16, tag=f"tg_{q}", bufs=2, name=f"tg_{q}_{l}") for l in range(2)],
                        stg=[wp.tile([H, 8], F32, tag=f"stg{q}{p}", bufs=1, name=f"stg{q}{p}") for p in range(2)],
                    )
                for i, b in enumerate(blocks):
                    q, hf = i // 2, i % 2
                    P = pairs[q]
                    sl_h = slice(hf * B, (hf + 1) * B)
                    nb = wp.tile([ND, B], F32R, tag="nb", bufs=2, name=f"nb{b}")
                    nc.sync.dma_start(out=nb, in_=node_t_ext[:, b * B:(b + 1) * B])
                    px = pp.tile([H, B], F32, tag="psg", bufs=2, name=f"px{b}")
                    for hh in range(NHB):
                        nc.tensor.matmul(px[:, hh * HB:(hh + 1) * HB], wencT_node,
                                         nb[:, hh * HB:(hh + 1) * HB], start=True, stop=True)
                    nc.vector.tensor_copy(P["enc"][:, sl_h], px)
                    for dst in (P["h1"][0], P["h2"][0]):
                        nc.vector.tensor_scalar_add(dst[:, sl_h], px, e0[:, blkmap[b]:blkmap[b] + 1])
                    for dst in (P["c1"][0], P["c2"][0]):
                        nc.scalar.activation(out=dst[:, sl_h], in_=px, func=AF.Identity,
                                             bias=e0s[:, blkmap[b]:blkmap[b] + 1], scale=S_C)
                    ctx[i] = dict(b=b, g=blkmap[b], q=q, hf=hf)

                def gates(i, l, whhT, wihT, hprev, xin, stg_p, k_in):
                    """4 gate psums -> sig/tanh results into pair-tile halves."""
                    q, hf = ctx[i]["q"], ctx[i]["hf"]
                    P = pairs[q]
                    sl_h = slice(hf * B, (hf + 1) * B)
                    dests = [P["si"][l], P["sf"][l], P["tg"][l], P["so"][l]]
                    for c in range(4):
                        pg = pp.tile([H, B], F32, tag="psg" if c == 2 else "ps2",
                                     bufs=2, name=f"g_{c}")
                        for hh in range(NHB):
                            sl = slice(hh * HB, (hh + 1) * HB)
                            nc.tensor.matmul(pg[:, sl], whhT[:, c * H:(c + 1) * H],
                                             hprev[:, sl], start=True, stop=False)
                        for hh in range(NHB):
                            sl = slice(hh * HB, (hh + 1) * HB)
                            nc.tensor.matmul(pg[:, sl], wihT[0:k_in, c * H:(c + 1) * H],
                                             xin[0:k_in, sl], start=False, stop=True)
                        a = dests[c][:, sl_h]
                        if c == 2:
                            nc.vector._custom_dve(TANH7B, out=a, in0=pg,
                                                  in1=stg_p[:, 2:3],
                                                  s0=B2_G, s1=B1_G, imm2=B0_G)
                        else:
                            nc.scalar.activation(out=a, in_=pg, func=GATE_SIG[c],
                                                 bias=stg_p[:, c:c + 1], scale=1.0)

                def stage1g(i, t, par):
                    q, hf = ctx[i]["q"], ctx[i]["hf"]
                    P = pairs[q]
                    pcur = par
                    if hf == 0:
                        nc.gpsimd.tensor_copy(P["stg"][pcur], m12[:, ctx[i]["g"], :, DS(t, 1)].squeeze(-1))
                    sl_h = slice(hf * B, (hf + 1) * B)
                    gates(i, 0, whhT0, wihT0, P["h1"][pcur][:, sl_h], P["enc"][:, sl_h],
                          P["stg"][pcur], H)

                def stage1c(q, par):
                    P = pairs[q]
                    pcur, pnxt = par, 1 - par
                    t1 = wp.tile([H, 2 * B], BF16, tag="prodB", bufs=4, name="t1")
                    t2 = wp.tile([H, 2 * B], BF16, tag="prodB", bufs=4, name="t2")
                    nc.vector.tensor_mul(t1, P["si"][0], P["tg"][0])
                    nc.vector.tensor_mul(t2, P["sf"][0], P["c1"][pcur])
                    nc.vector.tensor_add(P["c1"][pnxt], t1, t2)

                def stage1b(q, par):
                    P = pairs[q]
                    pnxt = 1 - par
                    tc1 = wp.tile([H, 2 * B], BF16, tag="actsB", bufs=4, name="tc1")
                    nc.vector._custom_dve(TANH7P, out=tc1, in0=P["c1"][pnxt],
                                          s0=B2_C, s1=B1_C, imm2=B0_C)
                    nc.vector.tensor_mul(P["h1"][pnxt], P["so"][0], tc1)

                def stage2g(i, par):
                    q, hf = ctx[i]["q"], ctx[i]["hf"]
                    P = pairs[q]
                    pcur, pnxt = par, 1 - par
                    sl_h = slice(hf * B, (hf + 1) * B)
                    stg2 = P["stg"][pcur][:, 4:]
                    gates(i, 1, whhT1, wihT1, P["h2"][pcur][:, sl_h],
                          P["h1"][pnxt][:, sl_h], stg2, H - GM)

                def stage2c(q, par):
                    P = pairs[q]
                    pcur, pnxt = par, 1 - par
                    t3 = wp.tile([H, 2 * B], BF16, tag="prodB", bufs=4, name="t3")
                    t4 = wp.tile([H, 2 * B], BF16, tag="prodB", bufs=4, name="t4")
                    nc.vector.tensor_mul(t3, P["si"][1], P["tg"][1])
                    nc.vector.tensor_mul(t4, P["sf"][1], P["c2"][pcur])
                    nc.vector.tensor_add(P["c2"][pnxt], t3, t4)

                def stage2b(q, par):
                    P = pairs[q]
                    pnxt = 1 - par
                    tc2 = wp.tile([H, 2 * B], BF16, tag="actsB", bufs=4, name="tc2")
                    nc.vector._custom_dve(TANH7P, out=tc2, in0=P["c2"][pnxt],
                                          s0=B2_C, s1=B1_C, imm2=B0_C)
                    nc.vector.tensor_mul(P["h2"][pnxt], P["so"][1], tc2)

                def stage3(i, th, par, half):
                    b = ctx[i]["b"]
                    q, hf = ctx[i]["q"], ctx[i]["hf"]
                    P = pairs[q]
                    h2 = P["h2"][1 - par][:, hf * B:(hf + 1) * B]
                    c2 = P["c2"][1 - par][:, hf * B:(hf + 1) * B]
                    pd = pp.tile([64, B], F32, tag="psg", bufs=2, name="pd")
                    for hh in range(NHB):
                        sl = slice(hh * HB, (hh + 1) * HB)
                        nc.tensor.matmul(pd[:, sl], wd1T[:, 0, :], h2[:, sl],
                                         start=True, stop=False)
                    for hh in range(NHB):
                        sl = slice(hh * HB, (hh + 1) * HB)
                        nc.tensor.matmul(pd[:, sl], wd1T[:, 1, :], c2[:, sl],
                                         start=False, stop=True)
                    relu = wp.tile([64, B], BF16, tag="relu", bufs=3, name="relu")
                    nc.scalar.activation(out=relu, in_=pd, func=AF.Relu,
                                         bias=bd1, scale=1.0)
                    pyf = pp.tile([H, B], F32, tag="ps2", bufs=2, name="pyf")
                    py = pyf[:, 0:4 * NCH]
                    for j in range(NCH):
                        nc.tensor.matmul(py[:, j * 4:(j + 1) * 4],
                                         relu[:, j * H:(j + 1) * H], wd2T,
                                         start=True, stop=True)
                    ybuf = wp.tile([H, NCH, GM], F32, tag="ybuf", bufs=4, name="ybuf")
                    nc.scalar.copy(
                        ybuf, py.rearrange("p (j four) -> p j four", four=4)[:, :, 0:GM])
                    nc.sync.dma_start(
                        out=y_ext[b * B:(b + 1) * B, DS(th + half * THALF, 1), :]
                            .rearrange("(j p) t g -> p j (t g)", p=H),
                        in_=ybuf)

                NB_ = len(blocks)

                def S1(s_expr, par, half):
                    for i in range(NB_):
                        stage1g(i, s_expr + half * THALF, par)
                    for q in range(NPAIR):
                        stage1c(q, par)
                    for q in range(NPAIR):
                        stage1b(q, par)

                def S2(par):
                    for i in range(NB_):
                        stage2g(i, par)
                    for q in range(NPAIR):
                        stage2c(q, par)
                    for q in range(NPAIR):
                        stage2b(q, par)

                def S3(th_expr, par, half):
                    for i in range(NB_):
                        stage3(i, th_expr, par, half)

                assert (THALF - 2 - P0) % UNROLL == 0 and P0 % 2 == 0
                for half in range(2):
                    for sp in range(P0):
                        S1(sp, sp % 2, half)
                        if sp >= 1: S2((sp - 1) % 2)
                        if sp >= 2: S3(sp - 2, sp % 2, half)
                    with tc.For_i(P0, THALF - 2, UNROLL,
                                  hint_engines=(mybir.EngineType.PE,
                                                mybir.EngineType.Activation,
                                                mybir.EngineType.DVE)) as tv:
                        for k in range(UNROLL):
                            par = k % 2            # (tv + k) % 2, tv even
                            S1(tv + k, par, half)
                            S2(1 - par)            # step tv+k-1
                            S3(tv + k - 2, par, half)  # step tv+k-2
                    for sp in range(THALF - 2, THALF):
                        S1(sp, sp % 2, half)
                        S2((sp - 1) % 2)
                        S3(sp - 2, sp % 2, half)
                    S2(1)
                    S3(THALF - 2, 0, half)
                    S3(THALF - 1, 1, half)

    nc.finalize()
    return nc


_CACHE = {}
_LAST_IN_MAPS = None


def _get_nc(NBLK, NTAB, blkmap):
    key = (NBLK, NTAB, tuple(blkmap))
    if key not in _CACHE:
        _CACHE[key] = build_nc(NBLK, NTAB, blkmap)
    return _CACHE[key]


def kernel(node, ptr, graph_time_series_behavior, ground_motions,
           W_enc, b_enc, W_ih, W_hh, b_ih, b_hh, W_d1, b_d1, W_d2, b_d2):
    node = np.asarray(node, np.float32)
    ptr = np.asarray(ptr, np.int64)
    lat = np.asarray(graph_time_series_behavior, np.float32)
    gms = np.asarray(ground_motions, np.float32)
    W_enc = np.asarray(W_enc, np.float32); b_enc_a = np.asarray(b_enc, np.float32)
    W_ih = np.array(W_ih, np.float32, copy=True); W_hh = np.array(W_hh, np.float32, copy=True)
    b_ih = np.array(b_ih, np.float32, copy=True); b_hh = np.array(b_hh, np.float32, copy=True)
    W_d1 = np.array(W_d1, np.float32, copy=True); b_d1_a = np.asarray(b_d1, np.float32)
    W_d2 = np.asarray(W_d2, np.float32); b_d2_a = np.asarray(b_d2, np.float32)

    # fold the tanh-poly monic normalization into the weights:
    #  - g-gate rows (2H:3H) of both layers scaled by s_G (device computes
    #    tanh via the monic poly on the pre-scaled preactivation)
    #  - cell state stored as c_hat = s_C * c  => head's c columns unscale
    for l in range(2):
        W_ih[l][2 * H:3 * H, :] *= S_G
        W_hh[l][2 * H:3 * H, :] *= S_G
        b_ih[l][2 * H:3 * H] *= S_G
        b_hh[l][2 * H:3 * H] *= S_G
    W_d1[:, H:] /= S_C

    N = node.shape[0]
    BS = lat.shape[0]
    gsizes = np.diff(ptr).astype(np.int64)
    assert gsizes.sum() == N

    gper = (BS + NCORES - 1) // NCORES
    core_graphs = [list(range(c * gper, min((c + 1) * gper, BS))) for c in range(NCORES)]
    NTAB = max(len(cg) for cg in core_graphs)
    core_blkmaps, core_nblk = [], []
    for cg in core_graphs:
        bm = []
        for slot, g in enumerate(cg):
            bm += [slot] * int((gsizes[g] + B - 1) // B)
        core_blkmaps.append(bm)
        core_nblk.append(len(bm))
    NBLK = max(core_nblk) if max(core_nblk) > 0 else 1
    for bm in core_blkmaps:
        bm += [0] * (NBLK - len(bm))
    if all(bm == core_blkmaps[0] for bm in core_blkmaps):
        blkmap = core_blkmaps[0]
        per_block_tabs = False
    else:
        blkmap = list(range(NBLK))
        NTAB = NBLK
        per_block_tabs = True

    NPAD = NBLK * B

    # host-precomputed per-(graph, t) tables (small); computed AFTER the
    # s_G scaling above so g-gate biases arrive pre-scaled too
    enc_mix = np.einsum("hk,gtk->gth", W_enc[:, ND:ND + H], lat) \
        + np.einsum("hk,gtk->gth", W_enc[:, ND + H:], gms) + b_enc_a
    m1 = np.einsum("rh,gth->gtr", W_ih[0], enc_mix) + (b_ih[0] + b_hh[0])
    m2 = np.einsum("rk,gtk->gtr", W_ih[1][:, H - GM:], gms) + (b_ih[1] + b_hh[1])
    m12_full = np.concatenate([m1.reshape(BS, T, 4, H), m2.reshape(BS, T, 4, H)], axis=2)
    m12_full = np.ascontiguousarray(m12_full.transpose(3, 0, 2, 1))  # [H, BS, 8, T]
    e0_full = np.ascontiguousarray(enc_mix[:, 0, :].T)               # [H, BS]

    weights_common = dict(
        wencT_node=np.ascontiguousarray(W_enc[:, :ND].T),
        wihT0=np.ascontiguousarray(W_ih[0].T).astype(ml_dtypes.bfloat16),
        whhT0=np.ascontiguousarray(W_hh[0].T).astype(ml_dtypes.bfloat16),
        wihT1=np.ascontiguousarray(W_ih[1].T).astype(ml_dtypes.bfloat16),
        whhT1=np.ascontiguousarray(W_hh[1].T).astype(ml_dtypes.bfloat16),
        wd1T=np.ascontiguousarray(np.stack([W_d1[:, :H].T, W_d1[:, H:].T], axis=1)).astype(ml_dtypes.bfloat16),
        wd2T=np.ascontiguousarray(np.concatenate([W_d2.T, np.zeros((64, 1), np.float32)], 1)).astype(ml_dtypes.bfloat16),
        bd1=b_d1_a.reshape(64, 1),
        bd2rep=np.ascontiguousarray(np.broadcast_to(b_d2_a, (H, GM))),
    )

    in_maps, core_index_maps = [], []
    for c, cg in enumerate(core_graphs):
        node_pad = np.zeros((NPAD, ND), np.float32)
        idx_map = np.full(NPAD, -1, np.int64)
        pos = 0
        for g in cg:
            s, e = int(ptr[g]), int(ptr[g + 1])
            n = e - s
            node_pad[pos:pos + n] = node[s:e]
            idx_map[pos:pos + n] = np.arange(s, e)
            pos += int((n + B - 1) // B) * B
        m12_c = np.zeros((H, NTAB, 8, T), np.float32)
        e0_c = np.zeros((H, NTAB), np.float32)
        if per_block_tabs:
            bi = 0
            for g in cg:
                for _ in range(int((gsizes[g] + B - 1) // B)):
                    m12_c[:, bi] = m12_full[:, g]
                    e0_c[:, bi] = e0_full[:, g]
                    bi += 1
        else:
            for slot, g in enumerate(cg):
                m12_c[:, slot] = m12_full[:, g]
                e0_c[:, slot] = e0_full[:, g]
        in_maps.append(dict(
            node_t=np.ascontiguousarray(node_pad.T),
            m12=m12_c.astype(ml_dtypes.bfloat16),
            e0=e0_c,
            e0s=(S_C * e0_c),
            **weights_common,
        ))
        core_index_maps.append(idx_map)

    global _LAST_IN_MAPS
    _LAST_IN_MAPS = in_maps
    nc = _get_nc(NBLK, NTAB, blkmap)
    res = run_bass_kernel_spmd(nc, in_maps, list(range(NCORES)))

    out = np.empty((N, T, GM), np.float32)
    for c in range(NCORES):
        y = np.asarray(res.results[c]["y"], dtype=np.float32)
        m = core_index_maps[c]
        valid = m >= 0
        out[m[valid]] = y[valid]
    out += b_d2_a
    return out


# revision 22
# speedup vs baseline: 1.0908x; 1.0625x over previous
"""Trainium2 Bass kernel for NodeTimeSeriesDecoder (per-node 2-layer LSTM over T=256).

Sharding: data-parallel over graphs across 8 cores (whole graphs -> contiguous
node blocks). Per core nodes are padded per-graph to multiples of B=1024 and
processed as independent per-block LSTM chains; NI chains are interleaved in
one time loop so all engines stay busy despite the per-chain serial dependency.

The activation engine (ScalarE) is the bottleneck (10 nonlinear passes per
block-step saturate it). This version moves all tanh evaluations (g-gates and
tanh(c)) onto the Vector engine as a fused custom DVE op: a monic degree-7 odd
polynomial (8 ALU stages exactly). The per-(graph,t) gate bias rides the op's
Src1 [P,1] broadcast; the polynomial's leading-coefficient normalization is
folded into the host-side g-gate weight rows (scale s_G) and into a rescaled
cell state c_hat = s_C * c (head weights unscale it). Sigmoid gates remain on
ScalarE; the f*c products run on the otherwise-idle GPSIMD engine.
"""
import sys
sys.path.insert(0, "/opt/trn_rl_repo")
import re
import numpy as np
import ml_dtypes
import concourse.bass as bass
import concourse.bacc as bacc
import concourse.tile as tile
from concourse import mybir
from concourse.bass_utils import run_bass_kernel_spmd


F32 = mybir.dt.float32
F32R = mybir.dt.float32r
BF16 = mybir.dt.bfloat16
AF = mybir.ActivationFunctionType
ALU = mybir.AluOpType
DS = bass.DynSlice

H = 128
T = 256
GM = 3
ND = 6
B = 1024
NCORES = 8
NI = 4        # interleaved independent block chains
UNROLL = 8
P0 = 6
THALF = T // 2
NCH = B // H

# ---- degree-7 odd minimax-ish tanh fits (weighted toward the data bulk) ----
# tanh(x) ~= x*(a0 + a1 x^2 + a2 x^4 + a3 x^6) on |x| <= B_fit
A_G = (0.980187350165812, -0.2583327462684275, 0.0475225918234586, -0.00347542814802112)   # B_fit=2.45 (gate preacts)
A_C = (0.9904704872618362, -0.2862214716966526, 0.06479055237507263, -0.0063245456932585955)  # B_fit=2.0 (cell state)


def _monic(a):
    # y = s*x; tanh(x) ~= (((b2 - u)*u + b1)*u + b0) * y, u = y^2
    s = float((-a[3]) ** (1.0 / 7.0))
    b0 = a[0] / s
    b1 = a[1] / s ** 3
    b2 = a[2] / s ** 5
    return s, float(b0), float(b1), float(b2)


S_C, B0_C, B1_C, B2_C = _monic(A_C)
# g-site poly outputs s_C * tanh(x) directly (so i*g products are plain muls)
_A_GS = tuple(S_C * a for a in A_G)
S_G, B0_G, B1_G, B2_G = _monic(_A_GS)

# ---- custom DVE ops: monic degree-7 tanh on pre-scaled input ---------------
# NTANH7B: out = poly7(Src0 + latch(Src1))  (per-partition bias, for g-gates)
# NTANH7P: out = poly7(Src0)                (for tanh(c_hat))
_TANH_OPS = None


def _register_op(DO, name, spec):
    for existing in DO.OPS:
        if existing.name == name:
            return existing
    op = DO.DveOp(name, spec, subdim=False, uops_sha={})
    DO.OPS.append(op)
    DO.CUSTOM_DVE_SPECS[op.name] = op.spec
    DO._SUB_OPCODE_FOR_NAME[op.name] = DO._CUSTOM_DVE_ROW_BASE + len(DO.OPS) - 1
    assert DO._SUB_OPCODE_FOR_NAME[op.name] < 0x20
    for ver in ("v3", "v4"):
        try:
            op.compile(ver)
        except ValueError as e:
            m = re.search(r'="([0-9a-f]+)"', str(e))
            if not m:
                raise
            op.uops_sha[ver] = m.group(1)
            DO._COMPILE_CACHE.pop((op.name, ver), None)
            op.compile(ver)
    return op


def _get_tanh_ops():
    global _TANH_OPS
    if _TANH_OPS is not None:
        return _TANH_OPS
    from concourse import dve_ops as DO
    from concourse.dve_spec import Spec, Src0, Src1, C0, C1, C2, sq, Latch

    def _refb(in0, in1, s0, s1, imm2):
        tt = in0.astype(np.float32) + in1
        uu = tt * tt
        return (((s0 - uu) * uu + s1) * uu + imm2) * tt

    def _refp(in0, s0, s1, imm2):
        tt = in0.astype(np.float32)
        uu = tt * tt
        return (((s0 - uu) * uu + s1) * uu + imm2) * tt

    tb = Src0 + Latch(Src1)
    ub = sq(tb)
    opb = _register_op(DO, "NTANH7B",
                       Spec(body=(((C0 - ub) * ub + C1) * ub + C2) * tb,
                            reference=_refb))
    tp = Src0
    up = sq(tp)
    opp = _register_op(DO, "NTANH7P",
                       Spec(body=(((C0 - up) * up + C1) * up + C2) * tp,
                            reference=_refp))
    _TANH_OPS = (opb, opp)
    return _TANH_OPS


def build_nc(NBLK, NTAB, blkmap):
    NPAD = NBLK * B
    TANH7B, TANH7P = _get_tanh_ops()
    nc = bacc.Bacc(None, target_bir_lowering=False)

    node_t_ext = nc.declare_dram_parameter("node_t", [ND, NPAD], F32R, isOutput=False)
    m12_ext = nc.declare_dram_parameter("m12", [H, NTAB, 8, T], BF16, isOutput=False)
    e0_ext = nc.declare_dram_parameter("e0", [H, NTAB], F32, isOutput=False)
    e0s_ext = nc.declare_dram_parameter("e0s", [H, NTAB], F32, isOutput=False)
    wencT_node_ext = nc.declare_dram_parameter("wencT_node", [ND, H], F32R, isOutput=False)
    wihT0_ext = nc.declare_dram_parameter("wihT0", [H, 4 * H], BF16, isOutput=False)
    whhT0_ext = nc.declare_dram_parameter("whhT0", [H, 4 * H], BF16, isOutput=False)
    wihT1_ext = nc.declare_dram_parameter("wihT1", [H, 4 * H], BF16, isOutput=False)
    whhT1_ext = nc.declare_dram_parameter("whhT1", [H, 4 * H], BF16, isOutput=False)
    wd1T_ext = nc.declare_dram_parameter("wd1T", [H, 2, 64], BF16, isOutput=False)
    wd2T_ext = nc.declare_dram_parameter("wd2T", [64, 4], BF16, isOutput=False)
    bd1_ext = nc.declare_dram_parameter("bd1", [64, 1], F32, isOutput=False)
    bd2rep_ext = nc.declare_dram_parameter("bd2rep", [H, GM], F32, isOutput=False)
    y_ext = nc.declare_dram_parameter("y", [NPAD, T, GM], F32, isOutput=True)

    GATE_SIG = {0: AF.Sigmoid, 1: AF.Sigmoid, 3: AF.Sigmoid}

    with tile.TileContext(nc) as tc:
        with tc.tile_pool(name="consts", bufs=1) as cp, \
             tc.tile_pool(name="work", bufs=1) as wp, \
             tc.tile_pool(name="ps", bufs=8, space="PSUM") as pp:

            wencT_node = cp.tile([ND, H], F32R)
            wihT0 = cp.tile([H, 4 * H], BF16)
            whhT0 = cp.tile([H, 4 * H], BF16)
            wihT1 = cp.tile([H, 4 * H], BF16)
            whhT1 = cp.tile([H, 4 * H], BF16)
            wd1T = cp.tile([H, 2, 64], BF16)
            wd2T = cp.tile([64, 4], BF16)
            bd1 = cp.tile([64, 1], F32)
            bd2rep = cp.tile([H, GM], F32)
            m12 = cp.tile([H, NTAB, 8, T], BF16)
            e0 = cp.tile([H, NTAB], F32)
            e0s = cp.tile([H, NTAB], F32)
            zcol = cp.tile([H, 1], F32)
            nc.vector.memset(zcol, 0.0)
            for dst, src in [(wencT_node, wencT_node_ext), (wihT0, wihT0_ext),
                             (whhT0, whhT0_ext), (wihT1, wihT1_ext), (whhT1, whhT1_ext),
                             (wd1T, wd1T_ext), (wd2T, wd2T_ext), (bd1, bd1_ext),
                             (bd2rep, bd2rep_ext), (m12, m12_ext), (e0, e0_ext),
                             (e0s, e0s_ext)]:
                nc.sync.dma_start(out=dst, in_=src[:])

            HB = 512  # matmul free-dim chunk
            NHB = B // HB

            for bg in range(0, NBLK, NI):
                blocks = list(range(bg, min(bg + NI, NBLK)))
                ctx = {}
                NPAIR = (len(blocks) + 1) // 2
                pairs = {}
                for q in range(NPAIR):
                    pairs[q] = dict(
                        enc=wp.tile([H, 2 * B], BF16, tag=f"encn{q}", bufs=1, name=f"encn{q}"),
                        h1=[wp.tile([H, 2 * B], BF16, tag=f"h1_{q}{p}", bufs=1, name=f"h1_{q}{p}") for p in range(2)],
                        c1=[wp.tile([H, 2 * B], BF16, tag=f"c1_{q}{p}", bufs=1, name=f"c1_{q}{p}") for p in range(2)],
                        h2=[wp.tile([H, 2 * B], BF16, tag=f"h2_{q}{p}", bufs=1, name=f"h2_{q}{p}") for p in range(2)],
                        c2=[wp.tile([H, 2 * B], BF16, tag=f"c2_{q}{p}", bufs=1, name=f"c2_{q}{p}") for p in range(2)],
                        si=[wp.tile([H, 2 * B], BF16, tag=f"si_{q}", bufs=2, name=f"si_{q}_{l}") for l in range(2)],
                        sf=[wp.tile([H, 2 * B], BF16, tag=f"sf_{q}", bufs=2, name=f"sf_{q}_{l}") for l in range(2)],
                        so=[wp.tile([H, 2 * B], BF16, tag=f"so_{q}", bufs=2, name=f"so_{q}_{l}") for l in range(2)],
                        tg=[wp.tile([H, 2 * B], BF16, tag=f"tg_{q}", bufs=2, name=f"tg_{q}_{l}") for l in range(2)],
                        stg=[wp.tile([H, 8], F32, tag=f"stg{q}{p}", bufs=1, name=f"stg{q}{p}") for p in range(2)],
                    )
                for i, b in enumerate(blocks):
                    q, hf = i // 2, i % 2
                    P = pairs[q]
                    sl_h = slice(hf * B, (hf + 1) * B)
                    nb = wp.tile([ND, B], F32R, tag="nb", bufs=2, name=f"nb{b}")
                    nc.sync.dma_start(out=nb, in_=node_t_ext[:, b * B:(b + 1) * B])
                    px = pp.tile([H, B], F32, tag="psg", bufs=2, name=f"px{b}")
                    for hh in range(NHB):
                        nc.tensor.matmul(px[:, hh * HB:(hh + 1) * HB], wencT_node,
                                         nb[:, hh * HB:(hh + 1) * HB], start=True, stop=True)
                    nc.vector.tensor_copy(P["enc"][:, sl_h], px)
                    for dst in (P["h1"][0], P["h2"][0]):
                        nc.vector.tensor_scalar_add(dst[:, sl_h], px, e0[:, blkmap[b]:blkmap[b] + 1])
                    for dst in (P["c1"][0], P["c2"][0]):
                        nc.scalar.activation(out=dst[:, sl_h], in_=px, func=AF.Identity,
                                             bias=e0s[:, blkmap[b]:blkmap[b] + 1], scale=S_C)
                    ctx[i] = dict(b=b, g=blkmap[b], q=q, hf=hf)

                def gates(i, l, whhT, wihT, hprev, xin, stg_p, k_in):
                    """4 gate psums -> sig/tanh results into pair-tile halves."""
                    q, hf = ctx[i]["q"], ctx[i]["hf"]
                    P = pairs[q]
                    sl_h = slice(hf * B, (hf + 1) * B)
                    dests = [P["si"][l], P["sf"][l], P["tg"][l], P["so"][l]]
                    for c in range(4):
                        pg = pp.tile([H, B], F32, tag="psg" if c == 2 else "ps2",
                                     bufs=2, name=f"g_{c}")
                        for hh in range(NHB):
                            sl = slice(hh * HB, (hh + 1) * HB)
                            nc.tensor.matmul(pg[:, sl], whhT[:, c * H:(c + 1) * H],
                                             hprev[:, sl], start=True, stop=False)
                        for hh in range(NHB):
                            sl = slice(hh * HB, (hh + 1) * HB)
                            nc.tensor.matmul(pg[:, sl], wihT[0:k_in, c * H:(c + 1) * H],
                                             xin[0:k_in, sl], start=False, stop=True)
                        a = dests[c][:, sl_h]
                        if c == 2:
                            nc.vector._custom_dve(TANH7B, out=a, in0=pg,
                                                  in1=stg_p[:, 2:3],
                                                  s0=B2_G, s1=B1_G, imm2=B0_G)
                        else:
                            nc.scalar.activation(out=a, in_=pg, func=GATE_SIG[c],
                                                 bias=stg_p[:, c:c + 1], scale=1.0)

                def stage1g(i, t, par):
                    q, hf = ctx[i]["q"], ctx[i]["hf"]
                    P = pairs[q]
                    pcur = par
                    if hf == 0:
                        nc.gpsimd.tensor_copy(P["stg"][pcur], m12[:, ctx[i]["g"], :, DS(t, 1)].squeeze(-1))
                    sl_h = slice(hf * B, (hf + 1) * B)
                    gates(i, 0, whhT0, wihT0, P["h1"][pcur][:, sl_h], P["enc"][:, sl_h],
                          P["stg"][pcur], H)

                def stage1c(q, par):
                    P = pairs[q]
                    pcur, pnxt = par, 1 - par
                    t1 = wp.tile([H, 2 * B], BF16, tag="prodB", bufs=4, name="t1")
                    t2 = wp.tile([H, 2 * B], BF16, tag="prodB", bufs=4, name="t2")
                    nc.vector.tensor_mul(t1, P["si"][0], P["tg"][0])
                    nc.vector.tensor_mul(t2, P["sf"][0], P["c1"][pcur])
                    nc.vector.tensor_add(P["c1"][pnxt], t1, t2)

                def stage1b(q, par):
                    P = pairs[q]
                    pnxt = 1 - par
                    tc1 = wp.tile([H, 2 * B], BF16, tag="actsB", bufs=4, name="tc1")
                    nc.vector._custom_dve(TANH7P, out=tc1, in0=P["c1"][pnxt],
                                          s0=B2_C, s1=B1_C, imm2=B0_C)
                    nc.vector.tensor_mul(P["h1"][pnxt], P["so"][0], tc1)

                def stage2g(i, par):
                    q, hf = ctx[i]["q"], ctx[i]["hf"]
                    P = pairs[q]
                    pcur, pnxt = par, 1 - par
                    sl_h = slice(hf * B, (hf + 1) * B)
                    stg2 = P["stg"][pcur][:, 4:]
                    gates(i, 1, whhT1, wihT1, P["h2"][pcur][:, sl_h],
                          P["h1"][pnxt][:, sl_h], stg2, H - GM)

                def stage2c(q, par):
                    P = pairs[q]
                    pcur, pnxt = par, 1 - par
                    t3 = wp.tile([H, 2 * B], BF16, tag="prodB", bufs=4, name="t3")
                    t4 = wp.tile([H, 2 * B], BF16, tag="prodB", bufs=4, name="t4")
                    nc.vector.tensor_mul(t3, P["si"][1], P["tg"][1])
                    nc.vector.tensor_mul(t4, P["sf"][1], P["c2"][pcur])
                    nc.vector.tensor_add(P["c2"][pnxt], t3, t4)

                def stage2b(q, par):
                    P = pairs[q]
                    pnxt = 1 - par
                    tc2 = wp.tile([H, 2 * B], BF16, tag="actsB", bufs=4, name="tc2")
                    nc.vector._custom_dve(TANH7P, out=tc2, in0=P["c2"][pnxt],
                                          s0=B2_C, s1=B1_C, imm2=B0_C)
                    nc.vector.tensor_mul(P["h2"][pnxt], P["so"][1], tc2)

                def stage3(i, th, par, half):
                    b = ctx[i]["b"]
                    q, hf = ctx[i]["q"], ctx[i]["hf"]
                    P = pairs[q]
                    h2 = P["h2"][1 - par][:, hf * B:(hf + 1) * B]
                    c2 = P["c2"][1 - par][:, hf * B:(hf + 1) * B]
                    pd = pp.tile([64, B], F32, tag="psg", bufs=2, name="pd")
                    for hh in range(NHB):
                        sl = slice(hh * HB, (hh + 1) * HB)
                        nc.tensor.matmul(pd[:, sl], wd1T[:, 0, :], h2[:, sl],
                                         start=True, stop=False)
                    for hh in range(NHB):
                        sl = slice(hh * HB, (hh + 1) * HB)
                        nc.tensor.matmul(pd[:, sl], wd1T[:, 1, :], c2[:, sl],
                                         start=False, stop=True)
                    relu = wp.tile([64, B], BF16, tag="relu", bufs=3, name="relu")
                    nc.scalar.activation(out=relu, in_=pd, func=AF.Relu,
                                         bias=bd1, scale=1.0)
                    pyf = pp.tile([H, B], F32, tag="ps2", bufs=2, name="pyf")
                    py = pyf[:, 0:4 * NCH]
                    for j in range(NCH):
                        nc.tensor.matmul(py[:, j * 4:(j + 1) * 4],
                                         relu[:, j * H:(j + 1) * H], wd2T,
                                         start=True, stop=True)
                    ybuf = wp.tile([H, NCH, GM], F32, tag="ybuf", bufs=4, name="ybuf")
                    nc.scalar.copy(
                        ybuf, py.rearrange("p (j four) -> p j four", four=4)[:, :, 0:GM])
                    nc.sync.dma_start(
                        out=y_ext[b * B:(b + 1) * B, DS(th + half * THALF, 1), :]
                            .rearrange("(j p) t g -> p j (t g)", p=H),
                        in_=ybuf)

                NB_ = len(blocks)

                def S1(s_expr, par, half):
                    for i in range(NB_):
                        stage1g(i, s_expr + half * THALF, par)
                    for q in range(NPAIR):
                        stage1c(q, par)
                    for q in range(NPAIR):
                        stage1b(q, par)

                def S2(par):
                    for i in range(NB_):
                        stage2g(i, par)
                    for q in range(NPAIR):
                        stage2c(q, par)
                    for q in range(NPAIR):
                        stage2b(q, par)

                def S3(th_expr, par):
                    for i in range(NB_):
                        stage3(i, th_expr, par, 0)

                assert (T - 2 - P0) % UNROLL == 0 and P0 % 2 == 0
                for sp in range(P0):
                    S1(sp, sp % 2, 0)
                    if sp >= 1: S2((sp - 1) % 2)
                    if sp >= 2: S3(sp - 2, sp % 2)
                with tc.For_i(P0, T - 2, UNROLL,
                              hint_engines=(mybir.EngineType.PE,
                                            mybir.EngineType.Activation,
                                            mybir.EngineType.DVE)) as tv:
                    for k in range(UNROLL):
                        par = k % 2            # (tv + k) % 2, tv even
                        S1(tv + k, par, 0)
                        S2(1 - par)            # step tv+k-1
                        S3(tv + k - 2, par)    # step tv+k-2
                for sp in range(T - 2, T):
                    S1(sp, sp % 2, 0)
                    S2((sp - 1) % 2)
                    S3(sp - 2, sp % 2)
                S2(1)
                S3(T - 2, 0)
                S3(T - 1, 1)

    nc.finalize()
    return nc


_CACHE = {}
_LAST_IN_MAPS = None


def _get_nc(NBLK, NTAB, blkmap):
    key = (NBLK, NTAB, tuple(blkmap))
    if key not in _CACHE:
        _CACHE[key] = build_nc(NBLK, NTAB, blkmap)
    return _CACHE[key]


def kernel(node, ptr, graph_time_series_behavior, ground_motions,
           W_enc, b_enc, W_ih, W_hh, b_ih, b_hh, W_d1, b_d1, W_d2, b_d2):
    node = np.asarray(node, np.float32)
    ptr = np.asarray(ptr, np.int64)
    lat = np.asarray(graph_time_series_behavior, np.float32)
    gms = np.asarray(ground_motions, np.float32)
    W_enc = np.asarray(W_enc, np.float32); b_enc_a = np.asarray(b_enc, np.float32)
    W_ih = np.array(W_ih, np.float32, copy=True); W_hh = np.array(W_hh, np.float32, copy=True)
    b_ih = np.array(b_ih, np.float32, copy=True); b_hh = np.array(b_hh, np.float32, copy=True)
    W_d1 = np.array(W_d1, np.float32, copy=True); b_d1_a = np.asarray(b_d1, np.float32)
    W_d2 = np.asarray(W_d2, np.float32); b_d2_a = np.asarray(b_d2, np.float32)

    # fold the tanh-poly monic normalization into the weights:
    #  - g-gate rows (2H:3H) of both layers scaled by s_G (device computes
    #    tanh via the monic poly on the pre-scaled preactivation)
    #  - cell state stored as c_hat = s_C * c  => head's c columns unscale
    for l in range(2):
        W_ih[l][2 * H:3 * H, :] *= S_G
        W_hh[l][2 * H:3 * H, :] *= S_G
        b_ih[l][2 * H:3 * H] *= S_G
        b_hh[l][2 * H:3 * H] *= S_G
    W_d1[:, H:] /= S_C

    N = node.shape[0]
    BS = lat.shape[0]
    gsizes = np.diff(ptr).astype(np.int64)
    assert gsizes.sum() == N

    gper = (BS + NCORES - 1) // NCORES
    core_graphs = [list(range(c * gper, min((c + 1) * gper, BS))) for c in range(NCORES)]
    NTAB = max(len(cg) for cg in core_graphs)
    core_blkmaps, core_nblk = [], []
    for cg in core_graphs:
        bm = []
        for slot, g in enumerate(cg):
            bm += [slot] * int((gsizes[g] + B - 1) // B)
        core_blkmaps.append(bm)
        core_nblk.append(len(bm))
    NBLK = max(core_nblk) if max(core_nblk) > 0 else 1
    for bm in core_blkmaps:
        bm += [0] * (NBLK - len(bm))
    if all(bm == core_blkmaps[0] for bm in core_blkmaps):
        blkmap = core_blkmaps[0]
        per_block_tabs = False
    else:
        blkmap = list(range(NBLK))
        NTAB = NBLK
        per_block_tabs = True

    NPAD = NBLK * B

    # host-precomputed per-(graph, t) tables (small); computed AFTER the
    # s_G scaling above so g-gate biases arrive pre-scaled too
    enc_mix = np.einsum("hk,gtk->gth", W_enc[:, ND:ND + H], lat) \
        + np.einsum("hk,gtk->gth", W_enc[:, ND + H:], gms) + b_enc_a
    m1 = np.einsum("rh,gth->gtr", W_ih[0], enc_mix) + (b_ih[0] + b_hh[0])
    m2 = np.einsum("rk,gtk->gtr", W_ih[1][:, H - GM:], gms) + (b_ih[1] + b_hh[1])
    m12_full = np.concatenate([m1.reshape(BS, T, 4, H), m2.reshape(BS, T, 4, H)], axis=2)
    m12_full = np.ascontiguousarray(m12_full.transpose(3, 0, 2, 1))  # [H, BS, 8, T]
    e0_full = np.ascontiguousarray(enc_mix[:, 0, :].T)               # [H, BS]

    weights_common = dict(
        wencT_node=np.ascontiguousarray(W_enc[:, :ND].T),
        wihT0=np.ascontiguousarray(W_ih[0].T).astype(ml_dtypes.bfloat16),
        whhT0=np.ascontiguousarray(W_hh[0].T).astype(ml_dtypes.bfloat16),
        wihT1=np.ascontiguousarray(W_ih[1].T).astype(ml_dtypes.bfloat16),
        whhT1=np.ascontiguousarray(W_hh[1].T).astype(ml_dtypes.bfloat16),
        wd1T=np.ascontiguousarray(np.stack([W_d1[:, :H].T, W_d1[:, H:].T], axis=1)).astype(ml_dtypes.bfloat16),
        wd2T=np.ascontiguousarray(np.concatenate([W_d2.T, np.zeros((64, 1), np.float32)], 1)).astype(ml_dtypes.bfloat16),
        bd1=b_d1_a.reshape(64, 1),
        bd2rep=np.ascontiguousarray(np.broadcast_to(b_d2_a, (H, GM))),
    )

    in_maps, core_index_maps = [], []
    for c, cg in enumerate(core_graphs):
        node_pad = np.zeros((NPAD, ND), np.float32)
        idx_map = np.full(NPAD, -1, np.int64)
        pos = 0
        for g in cg:
            s, e = int(ptr[g]), int(ptr[g + 1])
            n = e - s
            node_pad[pos:pos + n] = node[s:e]
            idx_map[pos:pos + n] = np.arange(s, e)
            pos += int((n + B - 1) // B) * B
        m12_c = np.zeros((H, NTAB, 8, T), np.float32)
        e0_c = np.zeros((H, NTAB), np.float32)
        if per_block_tabs:
            bi = 0
            for g in cg:
                for _ in range(int((gsizes[g] + B - 1) // B)):
                    m12_c[:, bi] = m12_full[:, g]
                    e0_c[:, bi] = e0_full[:, g]
                    bi += 1
        else:
            for slot, g in enumerate(cg):
                m12_c[:, slot] = m12_full[:, g]
                e0_c[:, slot] = e0_full[:, g]
        in_maps.append(dict(
            node_t=np.ascontiguousarray(node_pad.T),
            m12=m12_c.astype(ml_dtypes.bfloat16),
            e0=e0_c,
            e0s=(S_C * e0_c),
            **weights_common,
        ))
        core_index_maps.append(idx_map)

    global _LAST_IN_MAPS
    _LAST_IN_MAPS = in_maps
    nc = _get_nc(NBLK, NTAB, blkmap)
    res = run_bass_kernel_spmd(nc, in_maps, list(range(NCORES)))

    out = np.empty((N, T, GM), np.float32)
    for c in range(NCORES):
        y = np.asarray(res.results[c]["y"], dtype=np.float32)
        m = core_index_maps[c]
        valid = m >= 0
        out[m[valid]] = y[valid]
    out += b_d2_a
    return out
